# revision 9
# baseline (speedup 1.0000x reference)
"""Causal self-attention (B=4, T=2048, C=1024, H=16) on 8 TRN2 NeuronCores.

Sharding: core = (batch, head-group) on a 4x2 grid. Each core computes the
attention output of 8 heads for one batch element plus its partial out-proj
(y^T = w_out_slice^T @ out_heads^T); the two head-groups of a batch are summed
on the host (the "out_proj all-reduce").

On-chip dataflow is fully transposed so no transposes are ever needed:
  qk^T  = w_qkv_slice^T @ x^T          (C on partitions)
  v     = x @ w_v_slice                (T on partitions, natural)
  S^T   = k_h @ q_h^T                  (k-positions on partitions)
  P^T   = exp(S^T) * causal_mask      (no max-subtraction: scores ~ N(0,1))
  outT  = [v|1]^T @ P^T                (ones column accumulates sum-of-exp)
  y^T   = w_out_slice^T @ (outT/sumexp)

Perf notes vs the first working version:
- diagonal key blocks only compute/exp the causally-live query span, which
  trims ~15% of the attention matmul + exp work;
- the 1/sumexp spread/gather runs on the DVE 32x32 stream-transpose instead
  of SBUF->SBUF DMA round trips, and both heads share one K=32 broadcast
  matmul, so the deferred normalization never stalls the in-order PE queue
  (the old per-j stall re-throttled the PE clock via HAM ~16x per kernel);
- normalized output is written straight into the OT tiles by the final DVE
  multiply (no DMA), y^T is stored as bf16, and the initial loads are spread
  over four engine queues in consumption order.
"""

import sys
import types

if "/opt/trn_rl_repo" not in sys.path:
    sys.path.insert(0, "/opt/trn_rl_repo")

import numpy as np


def _install_ntff_hook_shim():
    """antenv.axon_hooks is missing in this image; provide it so that
    run_bass_kernel_spmd(trace=True) can capture NTFF profiles."""
    if "antenv.axon_hooks" in sys.modules:
        return
    try:
        from trn_agent_boot.trn_boot import _ntff_profile_via_ctypes

        hook = _ntff_profile_via_ctypes("/opt/axon/libaxon_pjrt.so")
    except Exception:
        hook = None
    m = types.ModuleType("antenv.axon_hooks")
    m.get_axon_ntff_profile_hook = lambda: hook
    sys.modules["antenv.axon_hooks"] = m


_install_ntff_hook_shim()

import concourse.bass as bass  # noqa: E402
from concourse import bacc  # noqa: E402
import concourse.mybir as mybir  # noqa: E402
import concourse.tile as tile  # noqa: E402
from concourse.bass_utils import run_bass_kernel_spmd  # noqa: E402

BF16 = mybir.dt.bfloat16
F32 = mybir.dt.float32
NPBF16 = mybir.dt.np(BF16)
EXP = mybir.ActivationFunctionType.Exp

B, T, C = 4, 2048, 1024
H, DH = 16, 64
HC = 8           # heads per core
CK = C // 128    # 8 contraction chunks over C
TB = T // 128    # 16 key blocks / T row blocks
QC = T // 512    # 4 query chunks
SCALE = 1.0 / np.sqrt(DH)

TRACE = False          # set True (e.g. from test.py) to capture an NTFF profile
LAST_RESULT = None     # BassKernelResults of the last run (exec_time_ns etc.)

_CACHE = None


def _build():
    """Build + compile the single-core Bass program (SPMD across 8 cores).

    Every matmul uses the full 128x128 PE tile configuration: k^T is
    zero-padded per head to 128 contraction rows (even heads live in rows
    0-63, odd heads in rows 64-127, matching the packed q^T layout so the
    zero rows mask the other head), and the PV stationary is widened to 128
    columns (output rows 65-127 are don't-care).
    """
    nc = bacc.Bacc("TRN2", target_bir_lowering=False, debug=False, num_devices=8)

    xT = nc.dram_tensor("xT", [C, T], BF16, kind="ExternalInput")
    wqkv = nc.dram_tensor("wqkv", [C, 3 * 512], BF16, kind="ExternalInput")
    bqk = nc.dram_tensor("bqk", [128, CK], F32, kind="ExternalInput")
    wout = nc.dram_tensor("wout", [512, C], BF16, kind="ExternalInput")
    msk = nc.dram_tensor("msk", [128, 128], BF16, kind="ExternalInput")
    yT = nc.dram_tensor("yT", [C, T], BF16, kind="ExternalOutput")

    VROW = HC * 65 + 63  # v block row: 8 x (64 v-dims + ones) + stationary pad

    with tile.TileContext(nc) as tc:
        with (
            tc.tile_pool(name="persist", bufs=1) as pp,
            tc.tile_pool(name="sc", bufs=3, space="PSUM") as scp,
            tc.tile_pool(name="oa", bufs=2, space="PSUM") as oap,
            tc.tile_pool(name="pt", bufs=4) as ptp,
            tc.tile_pool(name="ocp", bufs=2) as ocpp,
            tc.tile_pool(name="yst", bufs=3) as yst,
        ):
            QT = [pp.tile([128, T], BF16, tag=f"qt{p}", name=f"qt{p}")
                  for p in range(4)]
            KP = [pp.tile([128, T], BF16, tag=f"kp{h}", name=f"kp{h}")
                  for h in range(HC)]
            OT = [pp.tile([128, T], BF16, tag=f"ot{p}", name=f"ot{p}")
                  for p in range(4)]
            VA = pp.tile([128, TB, VROW], BF16, tag="va")
            MASKD = pp.tile([128, 128], BF16, tag="maskd")
            WOUT = pp.tile([128, 4, C], BF16, tag="wout")
            BQK = pp.tile([128, CK], F32, tag="bqk")
            XT = pp.tile([128, CK, T], BF16, tag="xt")
            WQ = pp.tile([128, CK, 1536], BF16, tag="wq")
            # normalization scratch (stream-transpose spread / gather)
            SEL = pp.tile([64, 128], BF16, tag="sel")
            RC = pp.tile([64, 512], F32, tag="rc")
            T1 = pp.tile([64, 512], F32, tag="t1")
            RB = pp.tile([64, 512], F32, tag="rb")
            T2 = pp.tile([64, 512], F32, tag="t2")
            CB = [pp.tile([64, 512], BF16, tag=f"cb{i}", name=f"cb{i}")
                  for i in range(2)]

            # ---- initial loads: consumption order on the two HWDGE
            # queues (hot data); gpsimd SWDGE (~2us/issue) gets only the
            # cold tail tensors ----
            _ldq = [nc.sync, nc.scalar]
            _qi = [0]

            def ld(dst, src):
                _ldq[_qi[0] % 2].dma_start(dst, src)
                _qi[0] += 1

            for kc in range(CK):  # v weights first: v-proj starts earliest
                ld(WQ[:, kc, 1024:1536], wqkv[kc * 128:(kc + 1) * 128, 1024:1536])
            for n in range(4):
                for kc in range(CK):
                    ld(XT[:, kc, n * 512:(n + 1) * 512],
                       xT[kc * 128:(kc + 1) * 128, n * 512:(n + 1) * 512])
            for kc in range(CK):
                ld(WQ[:, kc, 0:512], wqkv[kc * 128:(kc + 1) * 128, 0:512])
                ld(WQ[:, kc, 512:1024], wqkv[kc * 128:(kc + 1) * 128, 512:1024])
            nc.gpsimd.dma_start(BQK[:], bqk[:])
            nc.gpsimd.dma_start(MASKD[:], msk[:])
            for kc in range(4):
                nc.gpsimd.dma_start(WOUT[:, kc, :], wout[kc * 128:(kc + 1) * 128, :])

            # ---- constant / scratch init ----
            # ones columns of VA (col 64 of each head block) + zero the pad
            vav = VA[:, :, 0:520].rearrange("p t (h c) -> p t h c", c=65)
            nc.vector.memset(vav[:, :, :, 64:65], 1.0)
            nc.vector.memset(VA[:, :, 520:583], 0.0)
            # SEL: broadcast selector for the K=64 norm matmul (head rows
            # live at partitions 0 and 32: engine APs need 32-aligned bases)
            nc.vector.memset(SEL[:], 0.0)
            nc.vector.memset(SEL[0:1, 0:64], 1.0)
            nc.vector.memset(SEL[32:33, 64:128], 1.0)
            # zero the transpose scratch so untouched lanes stay finite
            nc.vector.memset(RC[:], 0.0)
            nc.vector.memset(T1[:], 0.0)
            nc.vector.memset(RB[:], 0.0)
            nc.vector.memset(T2[:], 0.0)
            # zero halves of the padded k^T tiles (gpsimd: off the DVE)
            for h in range(HC):
                po = (h % 2) * 64
                nc.gpsimd.memset(KP[h][64 - po:128 - po, :], 0.0)

            def emit_v(psl, t):
                for kc in range(CK):
                    nc.tensor.matmul(
                        psl,
                        XT[:, kc, t * 128:(t + 1) * 128],
                        WQ[:, kc, 1024:1536],
                        start=(kc == 0),
                        stop=(kc == CK - 1),
                    )
                src = psl.rearrange("p (h c) -> p h c", c=64)
                dst = VA[:, t, 0:520].rearrange("p (h c) -> p h c", c=65)[:, :, 0:64]
                nc.vector.tensor_copy(dst, src)

            def emit_qk(psl, pair, qk, n):
                m = pair + 4 * qk  # wqkv column chunk (q: 0-3, k: 4-7)
                for kc in range(CK):
                    nc.tensor.matmul(
                        psl,
                        WQ[:, kc, m * 128:(m + 1) * 128],
                        XT[:, kc, n * 512:(n + 1) * 512],
                        start=(kc == 0),
                        stop=(kc == CK - 1),
                    )
                ns = slice(n * 512, (n + 1) * 512)
                if qk == 0:
                    nc.vector.tensor_scalar_add(
                        QT[pair][:, ns], psl, BQK[:, m:m + 1]
                    )
                else:
                    # split per head into the padded k^T tiles (lane-aligned)
                    nc.vector.tensor_scalar_add(
                        KP[2 * pair][0:64, ns], psl[0:64, :], BQK[0:64, m:m + 1]
                    )
                    nc.vector.tensor_scalar_add(
                        KP[2 * pair + 1][64:128, ns], psl[64:128, :],
                        BQK[64:128, m:m + 1],
                    )

            # ---- v projection (needed by every pair's PV matmuls) ----
            for t2 in range(0, TB, 2):
                t3 = scp.tile([128, 1024], F32, tag="sc", name="vps")
                emit_v(t3[:, 0:512], t2)
                emit_v(t3[:, 512:1024], t2 + 1)

            pending_norms = []
            _cb_ctr = [0]
            ygroups = []

            def norm_part1(pair, j, oacc0, oacc1):
                # sumexp rows (ACT: ordered before subsequent exps, so the
                # oacc slot is free before the next j's PV needs it)
                nc.vector.tensor_copy(RC[0:1, :], oacc0[64:65, :])
                nc.vector.tensor_copy(RC[32:33, :], oacc1[64:65, :])
                # spread 2x512 across 64 lanes, reciprocal, gather back
                nc.vector.transpose(T1[:], RC[:])
                t1v = T1.rearrange("p (b c) -> p b c", c=32)
                rbv = RB.rearrange("p (b c) -> p b c", c=32)
                nc.vector.reciprocal(rbv[:, :, 0:1], t1v[:, :, 0:1])
                nc.vector.transpose(T2[:], RB[:])
                cb = CB[_cb_ctr[0] % 2]
                _cb_ctr[0] += 1
                nc.vector.tensor_copy(cb[:], T2[:])
                # out rows stacked: h0 -> 0:64, h1 -> 64:128
                ocp = ocpp.tile([128, 512], F32, tag="ocp", name="ocp")
                nc.vector.tensor_copy(ocp[0:64, :], oacc0[0:64, :])
                nc.vector.tensor_copy(ocp[64:128, :], oacc1[0:64, :])
                return ocp, cb

            def norm_part2(pair, j, ocp, cb):
                # replicate 1/sumexp: rows 0/1 of cb -> partitions 0:64/64:128
                bc = scp.tile([128, 1024], F32, tag="sc", name="bc")
                nc.tensor.matmul(bc[:, 0:512], SEL[:], cb[:], start=True,
                                 stop=True)
                nc.vector.tensor_mul(
                    OT[pair][:, j * 512:(j + 1) * 512], ocp[:], bc[:, 0:512]
                )

            def flush_one():
                if pending_norms:
                    pending_norms.pop(0)()

            def flush_all():
                while pending_norms:
                    pending_norms.pop(0)()

            def emit_ygroup(n, mo2):
                t3 = scp.tile([128, 1024], F32, tag="sc", name="yps")
                for s in range(2):
                    mo = mo2 + s
                    psl = t3[:, s * 512:(s + 1) * 512]
                    for kc in range(4):
                        nc.tensor.matmul(
                            psl,
                            WOUT[:, kc, mo * 128:(mo + 1) * 128],
                            OT[kc][:, n * 512:(n + 1) * 512],
                            start=(kc == 0),
                            stop=(kc == 3),
                        )
                    ys = yst.tile([128, 512], BF16, tag="ys", name="ys")
                    nc.vector.tensor_copy(ys[:], psl)
                    nc.sync.dma_start(
                        yT[mo * 128:(mo + 1) * 128, n * 512:(n + 1) * 512],
                        ys[:],
                    )

            # ---- per head-pair: qk projection, then attention ----
            for pair in range(4):
                heads = (2 * pair, 2 * pair + 1)
                qkjobs = [(qk, n) for qk in range(2) for n in range(4)]
                for g0 in range(0, 8, 2):
                    if g0 == 4:
                        flush_all()  # previous pair's last-j norm
                    t3 = scp.tile([128, 1024], F32, tag="sc", name="qkps")
                    for s in range(2):
                        qk, n = qkjobs[g0 + s]
                        emit_qk(t3[:, s * 512:(s + 1) * 512], pair, qk, n)

                jorder = [3, 1, 2, 0] if pair == 3 else range(QC)
                for j in jorder:
                    nb = 4 * (j + 1)  # causal: key blocks 0..nb-1
                    oaccs = [
                        oap.tile([128, 512], F32, tag="oacc", name=f"oacc{s}")
                        for s in range(2)
                    ]
                    for i in range(nb):
                        if i == 3:
                            flush_one()  # previous j's deferred norm
                        if i >= 4 and ygroups:
                            ygroups.pop(0)()
                        d = i - 4 * j
                        sc = scp.tile([128, 1024], F32, tag="sc", name="sc")
                        pt = ptp.tile([128, 1024], BF16, tag="pt")
                        if d < 0:
                            # full (past) key block: both heads, 512 queries
                            for s, h in enumerate(heads):
                                nc.tensor.matmul(
                                    sc[:, s * 512:(s + 1) * 512],
                                    KP[h][:, i * 128:(i + 1) * 128],
                                    QT[pair][:, j * 512:(j + 1) * 512],
                                    start=True,
                                    stop=True,
                                )
                            nc.scalar.activation(pt[:], sc[:], EXP)
                            for s, h in enumerate(heads):
                                nc.tensor.matmul(
                                    oaccs[s],
                                    VA[:, i, h * 65:h * 65 + 128],
                                    pt[:, s * 512:(s + 1) * 512],
                                    start=(i == 0),
                                    stop=(i == nb - 1),
                                )
                        else:
                            # diagonal block: only the live query span.
                            # h1 lands at column 512 so each score matmul
                            # stays within one PSUM bank; the gap [L:512]
                            # is exp'd but never read.
                            L = 512 - 128 * d
                            qs = j * 512 + 128 * d
                            for s, h in enumerate(heads):
                                nc.tensor.matmul(
                                    sc[:, s * 512:s * 512 + L],
                                    KP[h][:, i * 128:(i + 1) * 128],
                                    QT[pair][:, qs:qs + L],
                                    start=True,
                                    stop=True,
                                )
                            nc.scalar.activation(
                                pt[:, 0:512 + L], sc[:, 0:512 + L], EXP
                            )
                            for s in range(2):
                                # queries sharing the key 128-block: first
                                # 128 of each head's live span
                                tri = pt[:, s * 512:s * 512 + 128]
                                nc.vector.tensor_mul(tri, tri, MASKD[:])
                            for s, h in enumerate(heads):
                                nc.tensor.matmul(
                                    oaccs[s][:, 128 * d:512],
                                    VA[:, i, h * 65:h * 65 + 128],
                                    pt[:, s * 512:s * 512 + L],
                                    start=(i == 0),
                                    stop=(i == nb - 1),
                                )
                    ocp, cb = norm_part1(pair, j, oaccs[0], oaccs[1])
                    pending_norms.append(
                        lambda pair=pair, j=j, ocp=ocp, cb=cb: norm_part2(
                            pair, j, ocp, cb
                        )
                    )
                    if pair == 3 and j < QC - 1:
                        # y-slice j completes once the pending norm above
                        # flushes (at i==3 of j+1); stage its groups into the
                        # i>=4 exp-wait filler slots of the next j iteration.
                        for mo2 in range(0, 8, 2):
                            ygroups.append(
                                lambda n=j, mo2=mo2: emit_ygroup(n, mo2)
                            )

            flush_all()
            while ygroups:
                ygroups.pop(0)()
            for mo2 in range(0, 8, 2):
                emit_ygroup(3, mo2)

    nc.compile()
    return nc


def _make_mask():
    p = np.arange(128)[:, None]
    f = np.arange(128)[None, :]
    return (p <= f).astype(np.float32).astype(NPBF16)


def kernel(x, w_qkv, b_qkv, w_out, b_out):
    global _CACHE, LAST_RESULT
    x = np.asarray(x, np.float32)
    w_qkv = np.asarray(w_qkv, np.float32)
    b_qkv = np.asarray(b_qkv, np.float32)
    w_out = np.asarray(w_out, np.float32)
    b_out = np.asarray(b_out, np.float32)

    if _CACHE is None:
        _CACHE = _build()
    nc = _CACHE

    mask = _make_mask()
    in_maps = []
    for core in range(8):
        b = core // 2
        g = core % 2
        sl = slice(g * 512, (g + 1) * 512)
        wq = w_qkv[:, 0:1024][:, sl] * SCALE
        wk = w_qkv[:, 1024:2048][:, sl]
        wv = w_qkv[:, 2048:3072][:, sl]
        wqkv_c = np.ascontiguousarray(
            np.concatenate([wq, wk, wv], axis=1).astype(NPBF16)
        )
        bq = b_qkv[0:1024][sl] * SCALE
        bk = b_qkv[1024:2048][sl]
        bqk_c = np.ascontiguousarray(
            np.concatenate([bq, bk]).reshape(CK, 128).T.astype(np.float32)
        )
        in_maps.append(
            {
                "xT": np.ascontiguousarray(x[b].T.astype(NPBF16)),
                "wqkv": wqkv_c,
                "bqk": bqk_c,
                "wout": np.ascontiguousarray(w_out[sl, :].astype(NPBF16)),
                "msk": mask,
            }
        )

    res = run_bass_kernel_spmd(nc, in_maps, core_ids=list(range(8)), trace=TRACE)
    LAST_RESULT = res

    bv = b_qkv[2048:3072]  # folded on host (zero in practice)
    out = np.empty((B, T, C), np.float32)
    for b in range(B):
        acc = res.results[2 * b]["yT"].astype(np.float32) + res.results[
            2 * b + 1
        ]["yT"].astype(np.float32)
        out[b] = acc.T + b_out[None, :]
    if np.any(bv):
        # v-bias contributes bv @ w_out to every position (softmax sums to 1)
        out += (np.concatenate([bv]) @ w_out)[None, None, :]
    return out


# revision 10
# speedup vs baseline: 1.0260x; 1.0260x over previous
"""Causal self-attention (B=4, T=2048, C=1024, H=16) on 8 TRN2 NeuronCores.

Sharding: core = (batch, head-group) on a 4x2 grid. Each core computes the
attention output of 8 heads for one batch element plus its partial out-proj
(y^T = w_out_slice^T @ out_heads^T); the two head-groups of a batch are summed
on the host (the "out_proj all-reduce").

On-chip dataflow is fully transposed so no transposes are ever needed:
  qk^T  = w_qkv_slice^T @ x^T          (C on partitions)
  v     = x @ w_v_slice                (T on partitions, natural)
  S^T   = k_h @ q_h^T                  (k-positions on partitions)
  P^T   = exp(S^T) * causal_mask      (no max-subtraction: scores ~ N(0,1))
  outT  = [v|1]^T @ P^T                (ones column accumulates sum-of-exp)
  y^T   = w_out_slice^T @ (outT/sumexp)

Perf notes vs the first working version:
- diagonal key blocks only compute/exp the causally-live query span, which
  trims ~15% of the attention matmul + exp work;
- the 1/sumexp spread/gather runs on the DVE 32x32 stream-transpose instead
  of SBUF->SBUF DMA round trips, and both heads share one K=32 broadcast
  matmul, so the deferred normalization never stalls the in-order PE queue
  (the old per-j stall re-throttled the PE clock via HAM ~16x per kernel);
- normalized output is written straight into the OT tiles by the final DVE
  multiply (no DMA), y^T is stored as bf16, and the initial loads are spread
  over four engine queues in consumption order.
"""

import sys
import types

if "/opt/trn_rl_repo" not in sys.path:
    sys.path.insert(0, "/opt/trn_rl_repo")

import numpy as np


def _install_ntff_hook_shim():
    """antenv.axon_hooks is missing in this image; provide it so that
    run_bass_kernel_spmd(trace=True) can capture NTFF profiles."""
    if "antenv.axon_hooks" in sys.modules:
        return
    try:
        from trn_agent_boot.trn_boot import _ntff_profile_via_ctypes

        hook = _ntff_profile_via_ctypes("/opt/axon/libaxon_pjrt.so")
    except Exception:
        hook = None
    m = types.ModuleType("antenv.axon_hooks")
    m.get_axon_ntff_profile_hook = lambda: hook
    sys.modules["antenv.axon_hooks"] = m


_install_ntff_hook_shim()

import concourse.bass as bass  # noqa: E402
from concourse import bacc  # noqa: E402
import concourse.mybir as mybir  # noqa: E402
import concourse.tile as tile  # noqa: E402
from concourse.bass_utils import run_bass_kernel_spmd  # noqa: E402

BF16 = mybir.dt.bfloat16
F32 = mybir.dt.float32
NPBF16 = mybir.dt.np(BF16)
EXP = mybir.ActivationFunctionType.Exp

B, T, C = 4, 2048, 1024
H, DH = 16, 64
HC = 8           # heads per core
CK = C // 128    # 8 contraction chunks over C
TB = T // 128    # 16 key blocks / T row blocks
QC = T // 512    # 4 query chunks
SCALE = 1.0 / np.sqrt(DH)

TRACE = False          # set True (e.g. from test.py) to capture an NTFF profile
LAST_RESULT = None     # BassKernelResults of the last run (exec_time_ns etc.)

_CACHE = None


def _build():
    """Build + compile the single-core Bass program (SPMD across 8 cores).

    Every matmul uses the full 128x128 PE tile configuration: k^T is
    zero-padded per head to 128 contraction rows (even heads live in rows
    0-63, odd heads in rows 64-127, matching the packed q^T layout so the
    zero rows mask the other head), and the PV stationary is widened to 128
    columns (output rows 65-127 are don't-care).
    """
    nc = bacc.Bacc("TRN2", target_bir_lowering=False, debug=False, num_devices=8)

    xT = nc.dram_tensor("xT", [C, T], BF16, kind="ExternalInput")
    wqkv = nc.dram_tensor("wqkv", [C, 3 * 512], BF16, kind="ExternalInput")
    bqk = nc.dram_tensor("bqk", [128, CK], F32, kind="ExternalInput")
    wout = nc.dram_tensor("wout", [512, C], BF16, kind="ExternalInput")
    msk = nc.dram_tensor("msk", [128, 128], BF16, kind="ExternalInput")
    yT = nc.dram_tensor("yT", [C, T], BF16, kind="ExternalOutput")

    VROW = HC * 65 + 63  # v block row: 8 x (64 v-dims + ones) + stationary pad

    with tile.TileContext(nc) as tc:
        with (
            tc.tile_pool(name="persist", bufs=1) as pp,
            tc.tile_pool(name="sc", bufs=3, space="PSUM") as scp,
            tc.tile_pool(name="oa", bufs=2, space="PSUM") as oap,
            tc.tile_pool(name="pt", bufs=4) as ptp,
            tc.tile_pool(name="ocp", bufs=2) as ocpp,
            tc.tile_pool(name="yst", bufs=3) as yst,
        ):
            QT = [pp.tile([128, T], BF16, tag=f"qt{p}", name=f"qt{p}")
                  for p in range(4)]
            KP = [pp.tile([128, T], BF16, tag=f"kp{h}", name=f"kp{h}")
                  for h in range(HC)]
            OT = [pp.tile([128, T], BF16, tag=f"ot{p}", name=f"ot{p}")
                  for p in range(4)]
            VA = pp.tile([128, TB, VROW], BF16, tag="va")
            MASKD = pp.tile([128, 128], BF16, tag="maskd")
            WOUT = pp.tile([128, 4, C], BF16, tag="wout")
            BQK = pp.tile([128, CK], F32, tag="bqk")
            XT = pp.tile([128, CK, T], BF16, tag="xt")
            WQ = pp.tile([128, CK, 1536], BF16, tag="wq")
            # normalization scratch (stream-transpose spread / gather)
            SEL = pp.tile([64, 128], BF16, tag="sel")
            RC = pp.tile([64, 512], F32, tag="rc")
            T1 = pp.tile([64, 512], F32, tag="t1")
            RB = pp.tile([64, 512], F32, tag="rb")
            T2 = pp.tile([64, 512], F32, tag="t2")
            CB = [pp.tile([64, 512], BF16, tag=f"cb{i}", name=f"cb{i}")
                  for i in range(2)]

            # ---- initial loads: consumption order, 3 issue queues ----
            _ldq = [nc.sync, nc.scalar, nc.gpsimd]
            _qi = [0]

            def ld(dst, src):
                _ldq[_qi[0] % 3].dma_start(dst, src)
                _qi[0] += 1

            for kc in range(CK):  # v weights first: v-proj starts earliest
                ld(WQ[:, kc, 1024:1536], wqkv[kc * 128:(kc + 1) * 128, 1024:1536])
            for n in range(4):
                for kc in range(CK):
                    ld(XT[:, kc, n * 512:(n + 1) * 512],
                       xT[kc * 128:(kc + 1) * 128, n * 512:(n + 1) * 512])
            for kc in range(CK):
                ld(WQ[:, kc, 0:512], wqkv[kc * 128:(kc + 1) * 128, 0:512])
                ld(WQ[:, kc, 512:1024], wqkv[kc * 128:(kc + 1) * 128, 512:1024])
            ld(BQK[:], bqk[:])
            ld(MASKD[:], msk[:])
            for kc in range(4):
                ld(WOUT[:, kc, :], wout[kc * 128:(kc + 1) * 128, :])

            # ---- constant / scratch init ----
            # ones columns of VA (col 64 of each head block) + zero the pad
            vav = VA[:, :, 0:520].rearrange("p t (h c) -> p t h c", c=65)
            nc.vector.memset(vav[:, :, :, 64:65], 1.0)
            nc.vector.memset(VA[:, :, 520:583], 0.0)
            # SEL: broadcast selector for the K=64 norm matmul (head rows
            # live at partitions 0 and 32: engine APs need 32-aligned bases)
            nc.vector.memset(SEL[:], 0.0)
            nc.vector.memset(SEL[0:1, 0:64], 1.0)
            nc.vector.memset(SEL[32:33, 64:128], 1.0)
            # zero the transpose scratch so untouched lanes stay finite
            nc.vector.memset(RC[:], 0.0)
            nc.vector.memset(T1[:], 0.0)
            nc.vector.memset(RB[:], 0.0)
            nc.vector.memset(T2[:], 0.0)
            # zero halves of the padded k^T tiles (gpsimd: off the DVE)
            for h in range(HC):
                po = (h % 2) * 64
                nc.gpsimd.memset(KP[h][64 - po:128 - po, :], 0.0)

            # HAM warm-up: the PE has no real work until the first ~15us of
            # DMA ramp completes; junk matmuls keep the clock gate at 2.4GHz
            # so the projections start warm.
            warm = scp.tile([128, 1024], F32, tag="sc", name="warm")
            for w in range(200):
                nc.tensor.matmul(warm[:, 0:128], SEL[:], SEL[:],
                                 start=True, stop=True)

            def emit_v(psl, t):
                for kc in range(CK):
                    nc.tensor.matmul(
                        psl,
                        XT[:, kc, t * 128:(t + 1) * 128],
                        WQ[:, kc, 1024:1536],
                        start=(kc == 0),
                        stop=(kc == CK - 1),
                    )
                src = psl.rearrange("p (h c) -> p h c", c=64)
                dst = VA[:, t, 0:520].rearrange("p (h c) -> p h c", c=65)[:, :, 0:64]
                nc.vector.tensor_copy(dst, src)

            def emit_qk(psl, pair, qk, n):
                m = pair + 4 * qk  # wqkv column chunk (q: 0-3, k: 4-7)
                for kc in range(CK):
                    nc.tensor.matmul(
                        psl,
                        WQ[:, kc, m * 128:(m + 1) * 128],
                        XT[:, kc, n * 512:(n + 1) * 512],
                        start=(kc == 0),
                        stop=(kc == CK - 1),
                    )
                ns = slice(n * 512, (n + 1) * 512)
                if qk == 0:
                    nc.vector.tensor_scalar_add(
                        QT[pair][:, ns], psl, BQK[:, m:m + 1]
                    )
                else:
                    # split per head into the padded k^T tiles (lane-aligned)
                    nc.vector.tensor_scalar_add(
                        KP[2 * pair][0:64, ns], psl[0:64, :], BQK[0:64, m:m + 1]
                    )
                    nc.vector.tensor_scalar_add(
                        KP[2 * pair + 1][64:128, ns], psl[64:128, :],
                        BQK[64:128, m:m + 1],
                    )

            # ---- v projection (needed by every pair's PV matmuls) ----
            for t2 in range(0, TB, 2):
                t3 = scp.tile([128, 1024], F32, tag="sc", name="vps")
                emit_v(t3[:, 0:512], t2)
                emit_v(t3[:, 512:1024], t2 + 1)

            pending_norms = []
            _cb_ctr = [0]
            ygroups = []

            def norm_part1(pair, j, oacc0, oacc1):
                # evacuate each oacc with ONE full copy (the PSUM slot is
                # released after just these two ops; the DVE copy cost is
                # free-dim-bound, so 128 rows cost the same as 64)
                f0 = ocpp.tile([128, 512], F32, tag="f0", name="f0")
                f1 = ocpp.tile([128, 512], F32, tag="f1", name="f1")
                nc.vector.tensor_copy(f0[:], oacc0[:])
                nc.vector.tensor_copy(f1[:], oacc1[:])
                # sumexp rows out of the SBUF copies, spread across 64
                # lanes via 32x32 stream-transpose, reciprocal, gather back
                nc.vector.tensor_copy(RC[0:1, :], f0[64:65, :])
                nc.vector.tensor_copy(RC[32:33, :], f1[64:65, :])
                nc.vector.transpose(T1[:], RC[:])
                t1v = T1.rearrange("p (b c) -> p b c", c=32)
                rbv = RB.rearrange("p (b c) -> p b c", c=32)
                nc.vector.reciprocal(rbv[:, :, 0:1], t1v[:, :, 0:1])
                nc.vector.transpose(T2[:], RB[:])
                cb = CB[_cb_ctr[0] % 2]
                _cb_ctr[0] += 1
                nc.vector.tensor_copy(cb[:], T2[:])
                return f0, f1, cb

            def norm_part2(pair, j, f0, f1, cb):
                # replicate 1/sumexp: rows 0/1 of cb -> partitions 0:64/64:128
                bc = scp.tile([128, 1024], F32, tag="sc", name="bc")
                nc.tensor.matmul(bc[:, 0:512], SEL[:], cb[:], start=True,
                                 stop=True)
                js = slice(j * 512, (j + 1) * 512)
                nc.vector.tensor_mul(OT[pair][0:64, js], f0[0:64, :],
                                     bc[0:64, 0:512])
                nc.vector.tensor_mul(OT[pair][64:128, js], f1[0:64, :],
                                     bc[64:128, 0:512])

            def flush_one():
                if pending_norms:
                    pending_norms.pop(0)()

            def flush_all():
                while pending_norms:
                    pending_norms.pop(0)()

            def emit_ygroup(n, mo2):
                t3 = scp.tile([128, 1024], F32, tag="sc", name="yps")
                for s in range(2):
                    mo = mo2 + s
                    psl = t3[:, s * 512:(s + 1) * 512]
                    for kc in range(4):
                        nc.tensor.matmul(
                            psl,
                            WOUT[:, kc, mo * 128:(mo + 1) * 128],
                            OT[kc][:, n * 512:(n + 1) * 512],
                            start=(kc == 0),
                            stop=(kc == 3),
                        )
                    ys = yst.tile([128, 512], BF16, tag="ys", name="ys")
                    nc.vector.tensor_copy(ys[:], psl)
                    nc.sync.dma_start(
                        yT[mo * 128:(mo + 1) * 128, n * 512:(n + 1) * 512],
                        ys[:],
                    )

            # ---- per head-pair: qk projection, then attention ----
            for pair in range(4):
                heads = (2 * pair, 2 * pair + 1)
                qkjobs = [(qk, n) for qk in range(2) for n in range(4)]
                for g0 in range(0, 8, 2):
                    if g0 == 4:
                        flush_all()  # previous pair's last-j norm
                    t3 = scp.tile([128, 1024], F32, tag="sc", name="qkps")
                    for s in range(2):
                        qk, n = qkjobs[g0 + s]
                        emit_qk(t3[:, s * 512:(s + 1) * 512], pair, qk, n)

                for j in range(QC):
                    nb = 4 * (j + 1)  # causal: key blocks 0..nb-1
                    oaccs = [
                        oap.tile([128, 512], F32, tag="oacc", name=f"oacc{s}")
                        for s in range(2)
                    ]
                    for i in range(nb):
                        if i == 3:
                            flush_one()  # previous j's deferred norm
                        if i >= 4 and ygroups:
                            ygroups.pop(0)()
                        d = i - 4 * j
                        sc = scp.tile([128, 1024], F32, tag="sc", name="sc")
                        pt = ptp.tile([128, 1024], BF16, tag="pt")
                        if d < 0:
                            # full (past) key block: both heads, 512 queries
                            for s, h in enumerate(heads):
                                nc.tensor.matmul(
                                    sc[:, s * 512:(s + 1) * 512],
                                    KP[h][:, i * 128:(i + 1) * 128],
                                    QT[pair][:, j * 512:(j + 1) * 512],
                                    start=True,
                                    stop=True,
                                )
                            nc.scalar.activation(pt[:], sc[:], EXP)
                            for s, h in enumerate(heads):
                                nc.tensor.matmul(
                                    oaccs[s],
                                    VA[:, i, h * 65:h * 65 + 128],
                                    pt[:, s * 512:(s + 1) * 512],
                                    start=(i == 0),
                                    stop=(i == nb - 1),
                                )
                        else:
                            # diagonal block: only the live query span.
                            # h1 lands at column 512 so each score matmul
                            # stays within one PSUM bank; the gap [L:512]
                            # is exp'd but never read.
                            L = 512 - 128 * d
                            qs = j * 512 + 128 * d
                            for s, h in enumerate(heads):
                                nc.tensor.matmul(
                                    sc[:, s * 512:s * 512 + L],
                                    KP[h][:, i * 128:(i + 1) * 128],
                                    QT[pair][:, qs:qs + L],
                                    start=True,
                                    stop=True,
                                )
                            nc.scalar.activation(
                                pt[:, 0:512 + L], sc[:, 0:512 + L], EXP
                            )
                            for s in range(2):
                                # queries sharing the key 128-block: first
                                # 128 of each head's live span
                                tri = pt[:, s * 512:s * 512 + 128]
                                nc.vector.tensor_mul(tri, tri, MASKD[:])
                            for s, h in enumerate(heads):
                                nc.tensor.matmul(
                                    oaccs[s][:, 128 * d:512],
                                    VA[:, i, h * 65:h * 65 + 128],
                                    pt[:, s * 512:s * 512 + L],
                                    start=(i == 0),
                                    stop=(i == nb - 1),
                                )
                    f0, f1, cb = norm_part1(pair, j, oaccs[0], oaccs[1])
                    pending_norms.append(
                        lambda pair=pair, j=j, f0=f0, f1=f1, cb=cb: norm_part2(
                            pair, j, f0, f1, cb
                        )
                    )
                    if pair == 3 and j < QC - 1:
                        # y-slice j completes once the pending norm above
                        # flushes (at i==3 of j+1); stage its groups into the
                        # i>=4 exp-wait filler slots of the next j iteration.
                        for mo2 in range(0, 8, 2):
                            ygroups.append(
                                lambda n=j, mo2=mo2: emit_ygroup(n, mo2)
                            )

            flush_all()
            while ygroups:
                ygroups.pop(0)()
            for mo2 in range(0, 8, 2):
                emit_ygroup(3, mo2)

    nc.compile()
    return nc


def _make_mask():
    p = np.arange(128)[:, None]
    f = np.arange(128)[None, :]
    return (p <= f).astype(np.float32).astype(NPBF16)


def kernel(x, w_qkv, b_qkv, w_out, b_out):
    global _CACHE, LAST_RESULT
    x = np.asarray(x, np.float32)
    w_qkv = np.asarray(w_qkv, np.float32)
    b_qkv = np.asarray(b_qkv, np.float32)
    w_out = np.asarray(w_out, np.float32)
    b_out = np.asarray(b_out, np.float32)

    if _CACHE is None:
        _CACHE = _build()
    nc = _CACHE

    mask = _make_mask()
    in_maps = []
    for core in range(8):
        b = core // 2
        g = core % 2
        sl = slice(g * 512, (g + 1) * 512)
        wq = w_qkv[:, 0:1024][:, sl] * SCALE
        wk = w_qkv[:, 1024:2048][:, sl]
        wv = w_qkv[:, 2048:3072][:, sl]
        wqkv_c = np.ascontiguousarray(
            np.concatenate([wq, wk, wv], axis=1).astype(NPBF16)
        )
        bq = b_qkv[0:1024][sl] * SCALE
        bk = b_qkv[1024:2048][sl]
        bqk_c = np.ascontiguousarray(
            np.concatenate([bq, bk]).reshape(CK, 128).T.astype(np.float32)
        )
        in_maps.append(
            {
                "xT": np.ascontiguousarray(x[b].T.astype(NPBF16)),
                "wqkv": wqkv_c,
                "bqk": bqk_c,
                "wout": np.ascontiguousarray(w_out[sl, :].astype(NPBF16)),
                "msk": mask,
            }
        )

    res = run_bass_kernel_spmd(nc, in_maps, core_ids=list(range(8)), trace=TRACE)
    LAST_RESULT = res

    bv = b_qkv[2048:3072]  # folded on host (zero in practice)
    out = np.empty((B, T, C), np.float32)
    for b in range(B):
        acc = res.results[2 * b]["yT"].astype(np.float32) + res.results[
            2 * b + 1
        ]["yT"].astype(np.float32)
        out[b] = acc.T + b_out[None, :]
    if np.any(bv):
        # v-bias contributes bv @ w_out to every position (softmax sums to 1)
        out += (np.concatenate([bv]) @ w_out)[None, None, :]
    return out
